# revision 1
# baseline (speedup 1.0000x reference)
"""Trainium2 Bass kernel for nn_Attention (linear attention + 1x1 convs + GroupNorm).

Math (per batch element, reference):
  qkv = W_qkv @ x            (1x1 conv, x: [512, 4096])
  q   = softmax_d(q) * scale ; k = softmax_n(k)
  ctx[h] = k_h @ v_h^T       (tiny [32,32] per head)
  att[h] = ctx[h]^T @ q_h    ([32, 4096])
  y   = W_out @ att + b      ; out = GroupNorm1(y) * gamma + beta

Kernel strategy (data parallel over batch, 2 batch elems per core):
  - all matmuls f32r (measured fastest on this hw; bf16 matmuls are ~2x
    slower in-kernel despite the cost model saying otherwise).
  - HBM traffic halved: x arrives bf16 and is upcast to f32 during the
    SWDGE (gpsimd) DMA; output stores are bf16 (host upcasts), ~2.8e-3 err.
  - q projection in standard layout [128(h d), n]; softmax-over-d denominator D
    via a block-diagonal ones matmul (PE), reciprocal on DVE; the q softmax
    division is applied in-place to eq per slice (accumulating row sums r').
  - k,v projections computed TRANSPOSED ([n, 128]) on PE with the x slice as
    stationary; k softmax denominator rides as ones-columns of v^T.
  - FOLD: ctx2 is transposed on PE (one 128-row transpose), then
    foldT = ctx2T-contraction with W_out^T in one 512-col matmul;
    the output projection consumes eq' directly -- the [128, 4096] att
    intermediate and its matmul pass are gone.
  - GroupNorm folds into the output epilogue: out = A[c]*(foldT.T@eq') + B[c]
    with stats from eq' only: S1 = <v0, r'> + C1; S2 = ||MM@eq'||_F^2 +
    2<v1, r'> + C2, where [MM^T | v0 | v1] = ctx2T-contraction with
    [L | colsum_W | W^T b] and G = W^T W = L L^T (host Cholesky).
"""

import numpy as np

B, C, HGT, WID = 16, 512, 64, 64
NSP = HGT * WID            # 4096 spatial
HEADS, DH = 4, 32
HID = HEADS * DH           # 128
NCORES = 8
BPC = B // NCORES          # 2 batch elems per core
SCALE = DH ** -0.5
EPS = 1e-5
SLICE = 512                # spatial slice for q/D/z/out matmuls
NSL = NSP // SLICE         # 8
KC = C // 128              # 4 contraction chunks
MC = C // 128              # 4 output-channel chunks
VTW = 132                  # v^T tile width: 128 v cols + 4 ones cols
LMW = 132                  # [L | u0 | u1 | pad2]

# pool buffer counts (tunable via env for experiments)
import os as _os
def _env(k, d):
    return int(_os.environ.get(k, d))
BUF_X = _env("KBUF_X", 3)
BUF_EQ = _env("KBUF_EQ", 2)
BUF_EKT = _env("KBUF_EKT", 2)
BUF_VTA = _env("KBUF_VTA", 2)
BUF_RD = _env("KBUF_RD", 2)
BUF_Y = _env("KBUF_Y", 4)
BUF_PSUM = _env("KBUF_PSUM", 4)
BUF_PSCD = _env("KBUF_PSCD", 3)
BUF_PSCTX = _env("KBUF_PSCTX", 1)
YSPLIT = _env("KYSPLIT", 1)  # 1: alternate output-copy between ACT and DVE
KSTORE = _env("KSTORE", 0)  # output store queue: 0=sync 1=gpsimd 2=scalar
KPIPE = _env("KPIPE", 1)  # emit phase E(b-1) after phase A(b); best steady-state
KXCAST = _env("KXCAST", 2)  # 1: x bf16 in HBM, DMA-cast to f32 on load
                            # 2: x bf16 in HBM+SBUF, bf16 q/kv matmuls
KCTXPAD = _env("KCTXPAD", 1)  # 1: pad ctx matmul moving width 132->256
KOBF16 = _env("KOBF16", 1)  # 1: output stored bf16

_CACHE = {}


def _build_nc(repeat=1):
    import concourse.bass as bass
    import concourse.mybir as mybir
    import concourse.tile as tile
    from concourse import bacc

    f32 = mybir.dt.float32
    f32r = mybir.dt.float32r
    bf16 = mybir.dt.bfloat16
    xhdt = bf16 if KXCAST else f32r
    xsdt = bf16 if KXCAST == 2 else f32r
    odt = bf16 if KOBF16 else f32
    AF = mybir.ActivationFunctionType
    OP = mybir.AluOpType
    AX = mybir.AxisListType

    nc = bacc.Bacc("TRN2", target_bir_lowering=False, debug=False)

    x_d = nc.dram_tensor("x", [BPC, C, NSP], xhdt, kind="ExternalInput")
    wq_d = nc.dram_tensor("wq_t", [C, HID], xsdt, kind="ExternalInput")
    wkv_d = nc.dram_tensor("wkv_t", [C, 2 * HID], xsdt, kind="ExternalInput")
    wout_d = nc.dram_tensor("wout_t", [HID, C], f32r, kind="ExternalInput")
    bones_d = nc.dram_tensor("b_ones", [HID, HID], f32r, kind="ExternalInput")
    mask_d = nc.dram_tensor("mask_scale", [HID, HID], f32, kind="ExternalInput")
    lmu_d = nc.dram_tensor("lmu", [HID, LMW], f32r, kind="ExternalInput")
    ident_d = nc.dram_tensor("ident", [HID, HID], f32r, kind="ExternalInput")
    ones_d = nc.dram_tensor("ones_col", [HID, 4], f32r, kind="ExternalInput")
    onesr_d = nc.dram_tensor("ones_row", [1, HID], f32r, kind="ExternalInput")
    vones_d = nc.dram_tensor("vones", [128, 32, VTW - 128], f32r, kind="ExternalInput")
    g4_d = nc.dram_tensor("gamma4", [128, MC], f32, kind="ExternalInput")
    gb4_d = nc.dram_tensor("gb4", [128, MC], f32, kind="ExternalInput")
    be4_d = nc.dram_tensor("beta4", [128, MC], f32, kind="ExternalInput")
    sc_d = nc.dram_tensor("sc", [1, 8], f32, kind="ExternalInput")
    out_d = nc.dram_tensor("out", [BPC, C, NSP], odt, kind="ExternalOutput")

    def r(ap):
        return ap.bitcast(f32r)

    with tile.TileContext(nc) as tc:
        with (
            tc.tile_pool(name="consts", bufs=1) as consts,
            tc.tile_pool(name="xp", bufs=BUF_X) as xp,
            tc.tile_pool(name="eqp", bufs=BUF_EQ) as eqp,
            tc.tile_pool(name="ektp", bufs=BUF_EKT) as ektp,
            tc.tile_pool(name="vtap", bufs=BUF_VTA) as vtap,
            tc.tile_pool(name="rdp", bufs=BUF_RD) as rdp,
            tc.tile_pool(name="foldp", bufs=2) as foldp,
            tc.tile_pool(name="yp", bufs=BUF_Y) as yp,
            tc.tile_pool(name="smalls", bufs=4) as smalls,
            tc.tile_pool(name="stp", bufs=2) as stp,
            tc.tile_pool(name="pp", bufs=BUF_PSUM, space="PSUM") as pp,
            tc.tile_pool(name="ppcd", bufs=BUF_PSCD, space="PSUM") as ppcd,
            tc.tile_pool(name="ppctx", bufs=BUF_PSCTX, space="PSUM") as ppctx,
        ):
            # --- constants: tiles up front; DMAs staged by first use ---
            wq_sb = consts.tile([128, KC, HID], xsdt)
            wkv_sb = consts.tile([128, KC, 2 * HID], xsdt)
            wout_sb = consts.tile([128, C], f32r)
            bones_sb = consts.tile([128, HID], f32r)
            mask_sb = consts.tile([128, HID], f32)
            lmu_sb = consts.tile([128, LMW], f32r)
            ident_sb = consts.tile([128, HID], f32r)
            ones_sb = consts.tile([128, 4], f32r)
            onesr_sb = consts.tile([1, HID], f32r)
            g4_sb = consts.tile([128, MC], f32)
            gb4_sb = consts.tile([128, MC], f32)
            be4_sb = consts.tile([128, MC], f32)
            sc_sb = consts.tile([1, 8], f32)

            # needed by phase A (q/kv projections + D matmul)
            nc.gpsimd.dma_start(
                out=wq_sb, in_=wq_d.ap().rearrange("(cc p) m -> p cc m", p=128)
            )
            nc.gpsimd.dma_start(
                out=wkv_sb, in_=wkv_d.ap().rearrange("(cc p) m -> p cc m", p=128)
            )
            nc.gpsimd.dma_start(out=bones_sb, in_=bones_d.ap())

            x_eng = nc.gpsimd if KXCAST == 1 else nc.sync

            # prime the DMA pipe: first two x slices of the first batch
            pre_xs = {}
            _xa0 = x_d.ap()[0].rearrange("(cc p) n -> p cc n", p=128)
            for _j in range(2):
                pxs = xp.tile([128, KC, SLICE], xsdt, tag="xs", name="pxs")
                x_eng.dma_start(
                    out=pxs, in_=_xa0[:, :, _j * SLICE : (_j + 1) * SLICE]
                )
                pre_xs[(0, _j)] = pxs

            def mid_consts():
                # needed by phases B-E
                nc.gpsimd.dma_start(out=mask_sb, in_=mask_d.ap())
                nc.gpsimd.dma_start(out=lmu_sb, in_=lmu_d.ap())
                nc.gpsimd.dma_start(out=ident_sb, in_=ident_d.ap())
                nc.gpsimd.dma_start(out=ones_sb, in_=ones_d.ap())
                nc.gpsimd.dma_start(out=onesr_sb, in_=onesr_d.ap())
                nc.gpsimd.dma_start(out=sc_sb, in_=sc_d.ap())
                nc.gpsimd.dma_start(out=wout_sb, in_=wout_d.ap())
                nc.gpsimd.dma_start(out=g4_sb, in_=g4_d.ap())
                nc.gpsimd.dma_start(out=gb4_sb, in_=gb4_d.ap())
                nc.gpsimd.dma_start(out=be4_sb, in_=be4_d.ap())

            batch_seq = [bi for _ in range(repeat) for bi in range(BPC)]

            vtw_a = 256 if KCTXPAD else VTW

            def phase_a_init(b, i_b=-1):
                xa = x_d.ap()[b].rearrange("(cc p) n -> p cc n", p=128)
                eq = eqp.tile([128, NSP], f32r, name="eq")
                ekt = ektp.tile([128, 32, 128], f32r, name="ekt")
                vta = vtap.tile([128, 32, vtw_a], f32r, name="vta")
                nc.gpsimd.dma_start(out=vta[:, :, 128:VTW], in_=vones_d.ap())
                if KCTXPAD and i_b <= 1:
                    # zero the pad lanes once per pool slot (reused after)
                    nc.vector.memset(vta[:, :, VTW:vtw_a].bitcast(f32), 0.0)
                racc = smalls.tile([128, NSL], f32, tag="racc", bufs=2, name="racc")
                return {"xa": xa, "eq": eq, "ekt": ekt, "vta": vta,
                        "racc": racc, "i_b": i_b}

            def phase_a_slice(S, j):
                xa, eq, ekt, vta, racc = (
                    S["xa"], S["eq"], S["ekt"], S["vta"], S["racc"]
                )
                sl = slice(j * SLICE, (j + 1) * SLICE)
                if (S["i_b"], j) in pre_xs:
                    xs = pre_xs.pop((S["i_b"], j))
                else:
                    xs = xp.tile([128, KC, SLICE], xsdt, tag="xs", name="xs")
                    x_eng.dma_start(out=xs, in_=xa[:, :, sl])

                # q projection (standard layout) + exp
                psq = pp.tile([128, SLICE], f32, tag="ps", name="psq")
                for cc in range(KC):
                    nc.tensor.matmul(
                        psq[:, :],
                        wq_sb[:, cc, :],
                        xs[:, cc, :],
                        start=(cc == 0),
                        stop=(cc == KC - 1),
                    )
                nc.scalar.activation(out=eq[:, sl], in_=psq[:, :], func=AF.Exp)

                # softmax-over-d denominator, broadcast within head + recip;
                # then scale eq in place (q softmax) accumulating row sums r'
                psd = pp.tile([128, SLICE], f32, tag="ps", name="psd")
                nc.tensor.matmul(
                    psd[:, :], bones_sb[:, :], eq[:, sl], start=True, stop=True
                )
                recipd = rdp.tile([128, SLICE], f32, name="recipd")
                nc.vector.reciprocal_approx_fast(out=recipd, in_=psd[:, :])
                nc.vector.scalar_tensor_tensor(
                    out=eq[:, sl],
                    in0=eq[:, sl],
                    scalar=1.0,
                    in1=recipd,
                    op0=OP.bypass,
                    op1=OP.mult,
                    accum_out=racc[:, j : j + 1],
                )

                # k,v transposed projections: x sub-slices as stationary operand
                for h in range(2):
                    pskv = pp.tile([128, SLICE], f32, tag="ps", name="pskv")
                    for s_ in range(2):
                        isub = 2 * h + s_
                        for cc in range(KC):
                            nc.tensor.matmul(
                                pskv[:, s_ * 256 : (s_ + 1) * 256],
                                xs[:, cc, isub * 128 : (isub + 1) * 128],
                                wkv_sb[:, cc, :],
                                start=(cc == 0),
                                stop=(cc == KC - 1),
                            )
                    kvv = pskv[:, :].rearrange("p (s o) -> p s o", s=2)
                    a0 = 4 * j + 2 * h
                    nc.scalar.activation(
                        out=ekt[:, a0 : a0 + 2, :], in_=kvv[:, :, 0:128], func=AF.Exp
                    )
                    nc.vector.tensor_copy(
                        out=vta[:, a0 : a0 + 2, 0:128], in_=kvv[:, :, 128:256]
                    )

            def phase_bcd(b, S):
                eq, ekt, vta, racc = S["eq"], S["ekt"], S["vta"], S["racc"]
                zacc = smalls.tile([128, NSL], f32, tag="zacc", bufs=2, name="zacc")

                # context (accumulate over all 32 spatial chunks)
                psctx = ppctx.tile([128, SLICE], f32, name="psctx")
                for i in range(32):
                    nc.tensor.matmul(
                        psctx[:, 0:vtw_a],
                        ekt[:, i, :],
                        vta[:, i, :],
                        start=(i == 0),
                        stop=(i == 31),
                    )
                inv_sk = smalls.tile([128, 1], f32, name="inv_sk")
                nc.vector.reciprocal_approx_fast(out=inv_sk, in_=psctx[:, 128:129])
                ctx2 = smalls.tile([128, HID], f32r, tag="ctx2", bufs=2, name="ctx2")
                nc.vector.scalar_tensor_tensor(
                    out=ctx2,
                    in0=psctx[:, 0:HID],
                    scalar=inv_sk,
                    in1=mask_sb[:, :],
                    op0=OP.mult,
                    op1=OP.mult,
                )

                # transpose ctx2 on PE, then fold: foldT = ctx2T-contract wout,
                # [MM^T | v0 | v1] = ctx2T-contract [L | u0 | u1]
                psT = ppcd.tile([128, HID], f32r, tag="pscd", name="psT")
                nc.tensor.transpose(psT[:, :], ctx2[:, :], ident_sb[:, :])
                ctx2T = smalls.tile([128, HID], f32r, tag="ctx2T", bufs=2, name="ctx2T")
                nc.vector.tensor_copy(out=ctx2T, in_=psT[:, :])

                psf = ppcd.tile([128, SLICE], f32, tag="pscd", name="psf")
                nc.tensor.matmul(
                    psf[:, :], ctx2T[:, :], wout_sb[:, :], start=True, stop=True
                )
                foldT = foldp.tile([128, C], f32r, name="foldT")
                nc.scalar.activation(out=foldT, in_=psf[:, :], func=AF.Copy)

                psf2 = ppcd.tile([128, SLICE], f32, tag="pscd", name="psf2")
                nc.tensor.matmul(
                    psf2[:, 0:LMW], ctx2T[:, :], lmu_sb[:, :], start=True, stop=True
                )
                mzT = smalls.tile([128, HID], f32r, tag="mzT", bufs=2, name="mzT")
                nc.vector.tensor_copy(out=mzT, in_=psf2[:, 0:HID])
                v01 = smalls.tile([128, 4], f32r, tag="v01", bufs=2, name="v01")
                nc.vector.tensor_copy(out=v01, in_=psf2[:, HID : HID + 4])

                # z = MM @ eq'; accumulate ||z||^2 via in-place Square on PSUM
                for j in range(NSL):
                    sl = slice(j * SLICE, (j + 1) * SLICE)
                    psz = ppcd.tile([128, SLICE], f32, tag="pscd", name="psz")
                    nc.tensor.matmul(
                        psz[:, :], mzT[:, :], eq[:, sl], start=True, stop=True
                    )
                    nc.scalar.activation(
                        out=psz[:, :], in_=psz[:, :], func=AF.Square,
                        accum_out=zacc[:, j : j + 1],
                    )

                r_sb = smalls.tile([128, 1], f32, name="r_sb")
                nc.vector.tensor_reduce(out=r_sb, in_=racc, axis=AX.X, op=OP.add)
                zred = smalls.tile([128, 1], f32, name="zred")
                nc.vector.tensor_reduce(out=zred, in_=zacc, axis=AX.X, op=OP.add)
                # duplicate into even-width f32r stationary operands
                rr = smalls.tile([128, 2], f32r, name="rr")
                nc.vector.tensor_copy(out=rr[:, 0:1], in_=r_sb)
                nc.vector.tensor_copy(out=rr[:, 1:2], in_=r_sb)
                zz = smalls.tile([128, 2], f32r, name="zz")
                nc.vector.tensor_copy(out=zz[:, 0:1], in_=zred)
                nc.vector.tensor_copy(out=zz[:, 1:2], in_=zred)

                # partition-0 dots: psA[0, 0]=<v0, r'>, psA[0, 1]=<v1, r'>
                psA = ppcd.tile([128, SLICE], f32, tag="pscd", name="psA")
                nc.tensor.matmul(
                    psA[0:2, 0:4], rr[:, :], v01[:, :], start=True, stop=True
                )
                # psB[0, 0] = sum_p zred = ||z||^2 total
                psB = ppcd.tile([128, SLICE], f32, tag="pscd", name="psB")
                nc.tensor.matmul(
                    psB[0:2, 0:4], zz[:, :], ones_sb[:, :], start=True, stop=True
                )

                # scalar chain on partition 0 (fused):
                st = stp.tile([1, 16], f32, name="st")
                inv_n = 1.0 / (C * NSP)
                nc.vector.scalar_tensor_tensor(
                    out=st[:, 2:3], in0=psA[0:1, 0:1], scalar=inv_n,
                    in1=sc_sb[:, 4:5], op0=OP.mult, op1=OP.add,
                )
                nc.vector.scalar_tensor_tensor(
                    out=st[:, 0:1], in0=psA[0:1, 1:2], scalar=2.0 * inv_n,
                    in1=sc_sb[:, 5:6], op0=OP.mult, op1=OP.add,
                )
                nc.vector.scalar_tensor_tensor(
                    out=st[:, 1:2], in0=psB[0:1, 0:1], scalar=inv_n,
                    in1=st[:, 0:1], op0=OP.mult, op1=OP.add,
                )
                nc.vector.tensor_mul(out=st[:, 3:4], in0=st[:, 2:3], in1=st[:, 2:3])
                nc.vector.tensor_sub(out=st[:, 4:5], in0=st[:, 1:2], in1=st[:, 3:4])
                # rstd = rsqrt(se), DVE-only: bit-trick seed + 3 Newton steps
                # (avoids Ln/Sqrt ACT table-set switches; se > 0 always)
                i32 = mybir.dt.int32
                nc.vector.tensor_scalar(
                    out=st[:, 6:7].bitcast(i32), in0=st[:, 4:5].bitcast(i32),
                    scalar1=1, scalar2=None, op0=OP.arith_shift_right,
                )
                nc.vector.tensor_scalar(
                    out=st[:, 6:7].bitcast(i32), in0=st[:, 6:7].bitcast(i32),
                    scalar1=-1, scalar2=0x5F3759DF,
                    op0=OP.mult, op1=OP.add,
                )
                for it in range(3):
                    dst = st[:, 8:9] if it == 2 else st[:, 6:7]
                    nc.vector.tensor_mul(out=st[:, 10:11], in0=st[:, 6:7], in1=st[:, 6:7])
                    nc.vector.tensor_mul(out=st[:, 10:11], in0=st[:, 10:11], in1=st[:, 4:5])
                    nc.vector.tensor_scalar(
                        out=st[:, 10:11], in0=st[:, 10:11],
                        scalar1=-0.5, scalar2=1.5, op0=OP.mult, op1=OP.add,
                    )
                    nc.vector.tensor_mul(out=dst, in0=st[:, 6:7], in1=st[:, 10:11])
                nc.vector.tensor_mul(out=st[:, 9:10], in0=st[:, 2:3], in1=st[:, 8:9])

                # broadcast (rstd, mu*rstd) across partitions via a K=1 matmul
                stb = stp.tile([1, 2], f32r, name="stb")
                nc.vector.tensor_copy(out=stb, in_=st[:, 8:10])
                psb = ppcd.tile([128, SLICE], f32, tag="pscd", name="psb")
                nc.tensor.matmul(
                    psb[:, 0:2], onesr_sb[:, :], stb[:, :], start=True, stop=True
                )
                rb = smalls.tile([128, 2], f32, name="rb")
                nc.vector.tensor_copy(out=rb, in_=psb[:, 0:2])

                # A = gamma*rstd ; Bc = gb*rstd - gamma*(mu*rstd) + beta
                a4 = smalls.tile([128, MC], f32, tag="a4", bufs=2, name="a4")
                nc.vector.tensor_scalar_mul(out=a4, in0=g4_sb[:, :], scalar1=rb[:, 0:1])
                b4 = smalls.tile([128, MC], f32, tag="b4", bufs=2, name="b4")
                nc.vector.tensor_scalar_mul(out=b4, in0=gb4_sb[:, :], scalar1=rb[:, 0:1])
                t4 = smalls.tile([128, MC], f32, name="t4")
                nc.vector.tensor_scalar_mul(out=t4, in0=g4_sb[:, :], scalar1=rb[:, 1:2])
                nc.vector.tensor_sub(out=b4, in0=b4, in1=t4)
                nc.vector.tensor_add(out=b4, in0=b4, in1=be4_sb[:, :])
                S["foldT"] = foldT
                S["a4"] = a4
                S["b4"] = b4

            def phase_e(b, S):
                foldT, a4, b4 = S["foldT"], S["a4"], S["b4"]
                eq = S["eq"]
                oa = out_d.ap()[b].rearrange("(mm p) n -> p mm n", p=128)
                for j in range(NSL):
                    sl = slice(j * SLICE, (j + 1) * SLICE)
                    ys4 = yp.tile([128, MC, SLICE], odt, name="ys4")
                    for m in range(MC):
                        psy = ppcd.tile([128, SLICE], f32, tag="pscd", name="psy")
                        nc.tensor.matmul(
                            psy[:, :],
                            foldT[:, m * 128 : (m + 1) * 128],
                            eq[:, sl],
                            start=True,
                            stop=True,
                        )
                        if YSPLIT and (m % 2 == 1):
                            nc.vector.tensor_scalar(
                                out=ys4[:, m, :], in0=psy[:, :],
                                scalar1=a4[:, m : m + 1], scalar2=b4[:, m : m + 1],
                                op0=OP.mult, op1=OP.add,
                            )
                        else:
                            nc.scalar.activation(
                                out=ys4[:, m, :], in_=psy[:, :], func=AF.Identity,
                                bias=b4[:, m : m + 1], scale=a4[:, m : m + 1],
                            )
                    store_eng = [nc.sync, nc.gpsimd, nc.scalar][KSTORE]
                    store_eng.dma_start(out=oa[:, :, sl], in_=ys4)

            if KPIPE == 1:
                states = []
                mid_consts()
                for i_b, b in enumerate(batch_seq):
                    S = phase_a_init(b, i_b)
                    states.append(S)
                    for j in range(NSL):
                        phase_a_slice(S, j)
                    if i_b > 0:
                        phase_e(batch_seq[i_b - 1], states[i_b - 1])
                    phase_bcd(b, S)
                states and phase_e(batch_seq[-1], states[-1])
            else:
                mid_consts()
                for i_b, b in enumerate(batch_seq):
                    S = phase_a_init(b, i_b)
                    for j in range(NSL):
                        phase_a_slice(S, j)
                    phase_bcd(b, S)
                    phase_e(b, S)

    nc.compile()
    return nc


def _host_consts(W_qkv, W_out, b_out, gamma, beta):
    W_qkv = np.asarray(W_qkv, np.float32)
    W_out = np.asarray(W_out, np.float32)
    b_out = np.asarray(b_out, np.float32)
    gamma = np.asarray(gamma, np.float32)
    beta = np.asarray(beta, np.float32)

    c = {}
    c["wq_t"] = np.ascontiguousarray(W_qkv[:HID].T)                 # [512, 128]
    c["wkv_t"] = np.ascontiguousarray(W_qkv[HID : 3 * HID].T)       # [512, 256]
    if KXCAST == 2:
        import ml_dtypes

        c["wq_t"] = c["wq_t"].astype(ml_dtypes.bfloat16)
        c["wkv_t"] = c["wkv_t"].astype(ml_dtypes.bfloat16)
    c["wout_t"] = np.ascontiguousarray(W_out.T)                     # [128, 512]
    blk = np.kron(np.eye(HEADS, dtype=np.float32), np.ones((DH, DH), np.float32))
    c["b_ones"] = blk                                               # [128, 128]
    c["mask_scale"] = (blk * SCALE).astype(np.float32)
    G = (W_out.astype(np.float64).T @ W_out.astype(np.float64))
    try:
        L = np.linalg.cholesky(G)
    except np.linalg.LinAlgError:
        w_ev, V = np.linalg.eigh(G)
        L = V @ np.diag(np.sqrt(np.clip(w_ev, 0.0, None)))
    lmu = np.zeros((HID, LMW), np.float32)
    lmu[:, 0:HID] = L.astype(np.float32)
    lmu[:, HID] = W_out.sum(axis=0)
    lmu[:, HID + 1] = W_out.T @ b_out
    c["lmu"] = lmu                                                  # [128, 132]
    c["ident"] = np.eye(HID, dtype=np.float32)
    c["ones_col"] = np.ones((HID, 4), np.float32)
    c["ones_row"] = np.ones((1, HID), np.float32)
    c["vones"] = np.ones((128, 32, VTW - 128), np.float32)
    c["gamma4"] = np.ascontiguousarray(gamma.reshape(MC, 128).T)
    c["gb4"] = np.ascontiguousarray((gamma * b_out).reshape(MC, 128).T)
    c["beta4"] = np.ascontiguousarray(beta.reshape(MC, 128).T)
    ntot = float(C) * float(NSP)
    sc = np.zeros((1, 8), np.float32)
    c1 = NSP * float(b_out.astype(np.float64).sum())
    c2 = NSP * float((b_out.astype(np.float64) ** 2).sum())
    sc[0, 0] = c1
    sc[0, 1] = c2
    sc[0, 2] = EPS
    sc[0, 3] = 1.0 / ntot
    sc[0, 4] = c1 / ntot                         # C1*invN
    sc[0, 5] = c2 / ntot + EPS                   # C2*invN + eps
    c["sc"] = sc
    return c


def _host_x(x):
    """Full f32 x -> per-core [BPC, C, NSP] shards (bf16 if KXCAST)."""
    x = np.asarray(x, np.float32)
    xr = x.reshape(B, C, NSP)
    if KXCAST:
        import ml_dtypes

        xr = xr.astype(ml_dtypes.bfloat16)
    return [
        np.ascontiguousarray(xr[ci * BPC : (ci + 1) * BPC]) for ci in range(NCORES)
    ]


def kernel(x, W_qkv, W_out, b_out, gamma, beta):
    from concourse.bass_utils import run_bass_kernel_spmd

    x = np.asarray(x, np.float32)
    assert x.shape == (B, C, HGT, WID)
    if "nc" not in _CACHE:
        _CACHE["nc"] = _build_nc()
    nc = _CACHE["nc"]

    consts = _host_consts(W_qkv, W_out, b_out, gamma, beta)
    xs = _host_x(x)
    in_maps = []
    for ci in range(NCORES):
        m = {"x": xs[ci]}
        m.update(consts)
        in_maps.append(m)

    res = run_bass_kernel_spmd(nc, in_maps, core_ids=list(range(NCORES)))
    out = np.concatenate([np.asarray(r_["out"], np.float32) for r_ in res.results], axis=0)
    return np.ascontiguousarray(out.reshape(B, C, HGT, WID))



# revision 52
# speedup vs baseline: 1.4496x; 1.4496x over previous
"""Trainium2 Bass kernel for nn_Attention (linear attention + 1x1 convs + GroupNorm).

Math (per batch element, reference):
  qkv = W_qkv @ x            (1x1 conv, x: [512, 4096])
  q   = softmax_d(q) * scale ; k = softmax_n(k)
  ctx[h] = k_h @ v_h^T       (tiny [32,32] per head)
  att[h] = ctx[h]^T @ q_h    ([32, 4096])
  y   = W_out @ att + b      ; out = GroupNorm1(y) * gamma + beta

Kernel strategy (data parallel over batch, 2 batch elems per core):
  - x arrives bf16 (HBM traffic halved); q/kv projection matmuls bf16.
    Measured on hw: fp8 DoubleRow gives NO speedup over bf16 (cost model's
    0.5 cyc/row is wrong for this silicon), and e3m4 moving operands are
    slower than bf16 - so fp8 paths exist behind knobs but stay off.
  - q projection in standard layout [128(h d), n]; softmax-over-d denominator D
    via a block-diagonal ones matmul (PE), reciprocal on DVE; the q softmax
    division is applied in-place to eq per slice (accumulating row sums r').
    The D matmul is issued AFTER the kv matmuls (AORD): exp(q) on ACT then
    overlaps the kv matmuls, so the PE never stalls waiting on it.
  - k,v projections computed TRANSPOSED ([n, 128]) on PE with the x slice as
    stationary; k softmax denominator rides as ones-columns of v^T.
  - context einsum over 32 spatial chunks in bf16, unpadded 132-wide
    (f32r matmuls <256 wide run at 1/4 rate; bf16 has no width penalty).
  - FOLD: ctx2 is transposed on PE (one 128-row transpose), then
    foldT = ctx2T-contraction with W_out^T in one 512-col matmul;
    the output projection consumes eq' directly -- the [128, 4096] att
    intermediate and its matmul pass are gone.
  - GroupNorm folds into the output epilogue: out = A[c]*(foldT.T@eq') + B[c]
    with stats from eq' only: S1 = <v0, r'> + C1; S2 = ||MM@eq'||_F^2 +
    2<v1, r'> + C2, where [MM^T | v0 | v1] = ctx2T-contraction with
    [L | colsum_W | W^T b] and G = W^T W = L L^T (host Cholesky).
"""

import numpy as np

B, C, HGT, WID = 16, 512, 64, 64
NSP = HGT * WID            # 4096 spatial
HEADS, DH = 4, 32
HID = HEADS * DH           # 128
NCORES = 8
BPC = B // NCORES          # 2 batch elems per core
SCALE = DH ** -0.5
EPS = 1e-5
SLICE = 512                # spatial slice for q/D/z/out matmuls
NSL = NSP // SLICE         # 8
KC = C // 128              # 4 contraction chunks
MC = C // 128              # 4 output-channel chunks
VTW = 132                  # v^T tile width: 128 v cols + 4 ones cols
WS8 = 16.0                 # fp8 weight pre-scale

import os as _os

KNOBS = dict(
    BUF_X=4, BUF_EQ=2, BUF_EKT=2, BUF_VTA=2, BUF_RD=2, BUF_Y=4,
    BUF_PSUM=4, BUF_PSCD=3, BUF_PSCTX=1,
    YSPLIT=1,   # 1: alternate output-copy between ACT and DVE
    STORE=0,    # output store queue: 0=sync 1=gpsimd 2=scalar
    PIPE=1,     # emit phase E(b-1) after phase A(b); best steady-state
    XCAST=2,    # x bf16 in HBM+SBUF, bf16 q/kv matmuls (legacy 0/1 modes kept)
    CTXPAD=1,   # 1: pad f32r ctx matmul moving width 132->256
    OBF16=1,    # 1: output stored bf16
    CTXBF16=1,  # 1: ekt/vta bf16, unpadded 132-wide ctx matmul
    KV8=0,      # 1: kv projection single-fp8 DoubleRow (probe; adds x8 input)
    QKV8R=0,    # 1: q+kv projections 3-term residual fp8 DoubleRow
    X3=0,       # 1: x stored e3m4 in HBM (bf16 weights, mixed matmul)
    LMUPAD=1,   # 1: pad lmu matmul moving width 132->256
    AORD=1,     # 1: issue D matmul after kv matmuls (PE stays streaming)
    SCHED=0,    # 1: split phase E around BCD so PE stays busy through the
                #    BCD DVE chain (E head / ctx+fold+z / E tail / stats)
    EMAJOR=0,   # 1: out-proj m-major (8 consecutive matmuls reuse stationary)
    VTAPOOL=0,  # 1: vta copy on Pool engine instead of DVE
    NOSTORE=0,  # timing probe: skip output stores (breaks correctness)
    XONCE=0,    # timing probe: load x only for slice 0 (breaks correctness)
    XENG=0,     # 1: x loads issued from Pool (gpsimd) queue instead of sync
    EQBF=0,     # 1: eq stored bf16 (2-byte DVE mode for softmax scale)
    VTASPLIT=0, # 1: vta copy h0 on ACT, h1 on DVE
    PROBE=0,    # timing probes (break correctness): 1=no D/softmax 2=no kv
                # 3=no q 4=no z 5=no out-proj 6=no ctx/fold chain
)


def env_cfg():
    cfg = dict(KNOBS)
    for k in cfg:
        v = _os.environ.get("K" + k)
        if v is not None:
            cfg[k] = int(v)
    return cfg


_CACHE = {}


def _build_nc(repeat=1, cfg=None):
    import concourse.bass as bass
    import concourse.mybir as mybir
    import concourse.tile as tile
    from concourse import bacc

    cfg = dict(cfg or env_cfg())
    BUF_X = cfg["BUF_X"]; BUF_EQ = cfg["BUF_EQ"]; BUF_EKT = cfg["BUF_EKT"]
    BUF_VTA = cfg["BUF_VTA"]; BUF_RD = cfg["BUF_RD"]; BUF_Y = cfg["BUF_Y"]
    BUF_PSUM = cfg["BUF_PSUM"]; BUF_PSCD = cfg["BUF_PSCD"]; BUF_PSCTX = cfg["BUF_PSCTX"]
    YSPLIT = cfg["YSPLIT"]; KSTORE = cfg["STORE"]; KPIPE = cfg["PIPE"]
    KXCAST = cfg["XCAST"]; KCTXPAD = cfg["CTXPAD"]; KOBF16 = cfg["OBF16"]
    CTXBF16 = cfg["CTXBF16"]; KV8 = cfg["KV8"]; QKV8R = cfg["QKV8R"]
    X3 = cfg["X3"]; LMUPAD = cfg["LMUPAD"]
    AORD = cfg["AORD"]; SCHED = cfg["SCHED"]
    EMAJOR = cfg["EMAJOR"]; VTAPOOL = cfg["VTAPOOL"]
    NOSTORE = cfg["NOSTORE"]; XONCE = cfg["XONCE"]; XENG = cfg["XENG"]
    EQBF = cfg["EQBF"]; VTASPLIT = cfg["VTASPLIT"]; PROBE = cfg["PROBE"]

    f32 = mybir.dt.float32
    f32r = mybir.dt.float32r
    bf16 = mybir.dt.bfloat16
    f8e4 = mybir.dt.float8e4
    f8e5 = mybir.dt.float8e5
    f8e3 = mybir.dt.float8e3
    DR = mybir.MatmulPerfMode.DoubleRow

    HAS_X = not QKV8R            # bf16 (or e3m4) x input present
    HAS_X8 = KV8 or QKV8R        # e4m3 x input present
    HAS_DX = bool(QKV8R)         # e5m2 residual input present
    FP8KV = KV8 or QKV8R

    if X3:
        xhdt = f8e3
    elif KXCAST:
        xhdt = bf16
    else:
        xhdt = f32r
    xsdt = xhdt if (KXCAST == 2 or X3) else f32r
    wdt = bf16 if X3 else xsdt       # weights stay bf16 when x is e3m4
    cdt = bf16 if CTXBF16 else f32r  # ekt/vta dtype for ctx matmul
    odt = bf16 if KOBF16 else f32
    AF = mybir.ActivationFunctionType
    OP = mybir.AluOpType
    AX = mybir.AxisListType

    QSCL = (1.0 / WS8) if QKV8R else 1.0
    KSCL = (1.0 / WS8) if FP8KV else 1.0
    LMW = 256 if LMUPAD else 132

    nc = bacc.Bacc("TRN2", target_bir_lowering=False, debug=False)

    x_d = nc.dram_tensor("x", [BPC, C, NSP], xhdt, kind="ExternalInput") if HAS_X else None
    x8_d = nc.dram_tensor("x8", [BPC, C, NSP], f8e4, kind="ExternalInput") if HAS_X8 else None
    dx_d = nc.dram_tensor("dx5", [BPC, C, NSP], f8e5, kind="ExternalInput") if HAS_DX else None
    wq_d = nc.dram_tensor("wq_t", [C, HID], wdt, kind="ExternalInput") if not QKV8R else None
    wkv_d = nc.dram_tensor("wkv_t", [C, 2 * HID], wdt, kind="ExternalInput") if not FP8KV else None
    wq8_d = nc.dram_tensor("wq8_t", [C, HID], f8e4, kind="ExternalInput") if QKV8R else None
    dwq_d = nc.dram_tensor("dwq5_t", [C, HID], f8e5, kind="ExternalInput") if QKV8R else None
    wkv8_d = nc.dram_tensor("wkv8_t", [C, 2 * HID], f8e4, kind="ExternalInput") if FP8KV else None
    dwkv_d = nc.dram_tensor("dwkv5_t", [C, 2 * HID], f8e5, kind="ExternalInput") if QKV8R else None
    eqdt = bf16 if EQBF else f32r
    wout_d = nc.dram_tensor("wout_t", [HID, C], f32r, kind="ExternalInput")
    bones_d = nc.dram_tensor("b_ones", [HID, HID], eqdt, kind="ExternalInput")
    mask_d = nc.dram_tensor("mask_scale", [HID, HID], f32, kind="ExternalInput")
    lmu_d = nc.dram_tensor("lmu", [HID, LMW], f32r, kind="ExternalInput")
    ident_d = nc.dram_tensor("ident", [HID, HID], f32r, kind="ExternalInput")
    ones_d = nc.dram_tensor("ones_col", [HID, 4], f32r, kind="ExternalInput")
    onesr_d = nc.dram_tensor("ones_row", [1, HID], f32r, kind="ExternalInput")
    vones_d = nc.dram_tensor("vones", [128, 32, VTW - 128], cdt, kind="ExternalInput")
    g4_d = nc.dram_tensor("gamma4", [128, MC], f32, kind="ExternalInput")
    gb4_d = nc.dram_tensor("gb4", [128, MC], f32, kind="ExternalInput")
    be4_d = nc.dram_tensor("beta4", [128, MC], f32, kind="ExternalInput")
    sc_d = nc.dram_tensor("sc", [1, 8], f32, kind="ExternalInput")
    out_d = nc.dram_tensor("out", [BPC, C, NSP], odt, kind="ExternalOutput")

    with tile.TileContext(nc) as tc:
        with (
            tc.tile_pool(name="consts", bufs=1) as consts,
            tc.tile_pool(name="xp", bufs=BUF_X) as xp,
            tc.tile_pool(name="eqp", bufs=BUF_EQ) as eqp,
            tc.tile_pool(name="ektp", bufs=BUF_EKT) as ektp,
            tc.tile_pool(name="vtap", bufs=BUF_VTA) as vtap,
            tc.tile_pool(name="rdp", bufs=BUF_RD) as rdp,
            tc.tile_pool(name="foldp", bufs=2) as foldp,
            tc.tile_pool(name="yp", bufs=BUF_Y) as yp,
            tc.tile_pool(name="smalls", bufs=4) as smalls,
            tc.tile_pool(name="stp", bufs=2) as stp,
            tc.tile_pool(name="pp", bufs=BUF_PSUM, space="PSUM") as pp,
            tc.tile_pool(name="ppcd", bufs=BUF_PSCD, space="PSUM") as ppcd,
            tc.tile_pool(name="ppctx", bufs=BUF_PSCTX, space="PSUM") as ppctx,
        ):
            # --- constants: tiles up front; DMAs staged by first use ---
            wq_sb = consts.tile([128, KC, HID], wdt, name="wq_sb") if not QKV8R else None
            wkv_sb = consts.tile([128, KC, 2 * HID], wdt, name="wkv_sb") if not FP8KV else None
            wq8_sb = consts.tile([128, KC, HID], f8e4, name="wq8_sb") if QKV8R else None
            dwq_sb = consts.tile([128, KC, HID], f8e5, name="dwq_sb") if QKV8R else None
            wkv8_sb = consts.tile([128, KC, 2 * HID], f8e4, name="wkv8_sb") if FP8KV else None
            dwkv_sb = consts.tile([128, KC, 2 * HID], f8e5, name="dwkv_sb") if QKV8R else None
            wout_sb = consts.tile([128, C], f32r)
            bones_sb = consts.tile([128, HID], eqdt)
            mask_sb = consts.tile([128, HID], f32)
            lmu_sb = consts.tile([128, LMW], f32r)
            ident_sb = consts.tile([128, HID], f32r)
            ones_sb = consts.tile([128, 4], f32r)
            onesr_sb = consts.tile([1, HID], f32r)
            g4_sb = consts.tile([128, MC], f32)
            gb4_sb = consts.tile([128, MC], f32)
            be4_sb = consts.tile([128, MC], f32)
            sc_sb = consts.tile([1, 8], f32)

            def _wload(sb, d):
                nc.gpsimd.dma_start(
                    out=sb, in_=d.ap().rearrange("(cc p) m -> p cc m", p=128)
                )

            # needed by phase A (q/kv projections + D matmul)
            if wq_sb is not None:
                _wload(wq_sb, wq_d)
            if wkv_sb is not None:
                _wload(wkv_sb, wkv_d)
            if QKV8R:
                _wload(wq8_sb, wq8_d)
                _wload(dwq_sb, dwq_d)
                _wload(dwkv_sb, dwkv_d)
            if FP8KV:
                _wload(wkv8_sb, wkv8_d)
            nc.gpsimd.dma_start(out=bones_sb, in_=bones_d.ap())

            x_eng = nc.gpsimd if (KXCAST == 1 or XENG) else nc.sync

            def alloc_xtiles():
                t = {}
                if HAS_X:
                    xs_t = xp.tile([128, KC, SLICE], xsdt, tag="xs", name="xs")
                    t["xs"] = xs_t
                if HAS_X8:
                    x8_t = xp.tile([128, KC, SLICE], f8e4, tag="x8", name="x8")
                    t["x8"] = x8_t
                if HAS_DX:
                    dx_t = xp.tile([128, KC, SLICE], f8e5, tag="dx", name="dx")
                    t["dx"] = dx_t
                return t

            def dma_xtiles(t, b, j):
                sl = slice(j * SLICE, (j + 1) * SLICE)
                if "xs" in t:
                    a = x_d.ap()[b].rearrange("(cc p) n -> p cc n", p=128)
                    x_eng.dma_start(out=t["xs"], in_=a[:, :, sl])
                if "x8" in t:
                    a = x8_d.ap()[b].rearrange("(cc p) n -> p cc n", p=128)
                    x_eng.dma_start(out=t["x8"], in_=a[:, :, sl])
                if "dx" in t:
                    a = dx_d.ap()[b].rearrange("(cc p) n -> p cc n", p=128)
                    x_eng.dma_start(out=t["dx"], in_=a[:, :, sl])

            # prime the DMA pipe: first two x slices of the first batch
            pre_xs = {}
            for _j in range(2):
                t = alloc_xtiles()
                dma_xtiles(t, 0, _j)
                pre_xs[(0, _j)] = t

            def mid_consts():
                # needed by phases B-E
                nc.gpsimd.dma_start(out=mask_sb, in_=mask_d.ap())
                nc.gpsimd.dma_start(out=lmu_sb, in_=lmu_d.ap())
                nc.gpsimd.dma_start(out=ident_sb, in_=ident_d.ap())
                nc.gpsimd.dma_start(out=ones_sb, in_=ones_d.ap())
                nc.gpsimd.dma_start(out=onesr_sb, in_=onesr_d.ap())
                nc.gpsimd.dma_start(out=sc_sb, in_=sc_d.ap())
                nc.gpsimd.dma_start(out=wout_sb, in_=wout_d.ap())
                nc.gpsimd.dma_start(out=g4_sb, in_=g4_d.ap())
                nc.gpsimd.dma_start(out=gb4_sb, in_=gb4_d.ap())
                nc.gpsimd.dma_start(out=be4_sb, in_=be4_d.ap())

            batch_seq = [bi for _ in range(repeat) for bi in range(BPC)]

            if CTXBF16:
                vtw_a = VTW
            else:
                vtw_a = 256 if KCTXPAD else VTW

            def phase_a_init(b, i_b=-1):
                eq = eqp.tile([128, NSP], eqdt, name="eq")
                ekt = ektp.tile([128, 32, 128], cdt, name="ekt")
                vta = vtap.tile([128, 32, vtw_a], cdt, name="vta")
                nc.gpsimd.dma_start(out=vta[:, :, 128:VTW], in_=vones_d.ap())
                if (not CTXBF16) and KCTXPAD and i_b <= 1:
                    # zero the pad lanes once per pool slot (reused after)
                    nc.vector.memset(vta[:, :, VTW:vtw_a].bitcast(f32), 0.0)
                racc = smalls.tile([128, NSL], f32, tag="racc", bufs=2, name="racc")
                if PROBE == 1:
                    nc.vector.memset(racc, 0.0)
                if PROBE == 2:
                    nc.vector.memset(ekt.bitcast(f32), 0.0)
                    nc.vector.memset(vta[:, :, 0:128].bitcast(f32), 0.0)
                if PROBE == 3:
                    nc.vector.memset(eq.bitcast(f32), 0.0)
                return {"b": b, "eq": eq, "ekt": ekt, "vta": vta,
                        "racc": racc, "i_b": i_b}

            def phase_a_slice(S, j):
                eq, ekt, vta, racc = S["eq"], S["ekt"], S["vta"], S["racc"]
                sl = slice(j * SLICE, (j + 1) * SLICE)
                if XONCE:
                    if "xt" not in S:
                        if (S["i_b"], 0) in pre_xs:
                            S["xt"] = pre_xs.pop((S["i_b"], 0))
                        else:
                            S["xt"] = alloc_xtiles()
                            dma_xtiles(S["xt"], S["b"], 0)
                        pre_xs.pop((S["i_b"], 1), None)
                    t = S["xt"]
                elif (S["i_b"], j) in pre_xs:
                    t = pre_xs.pop((S["i_b"], j))
                else:
                    t = alloc_xtiles()
                    dma_xtiles(t, S["b"], j)

                # q projection (standard layout) + exp
                psq = None
                if PROBE != 3:
                    psq = pp.tile([128, SLICE], f32, tag="ps", name="psq")
                if PROBE == 3:
                    pass
                elif QKV8R:
                    groups = [(wq8_sb, t["x8"]), (wq8_sb, t["dx"]), (dwq_sb, t["x8"])]
                    n = 0
                    for wt, xt in groups:
                        for cc in (0, 2):
                            nc.tensor.matmul(
                                psq[:, :],
                                wt[:, cc : cc + 2, :],
                                xt[:, cc : cc + 2, :],
                                start=(n == 0),
                                stop=(n == 5),
                                perf_mode=DR,
                            )
                            n += 1
                else:
                    for cc in range(KC):
                        nc.tensor.matmul(
                            psq[:, :],
                            wq_sb[:, cc, :],
                            t["xs"][:, cc, :],
                            start=(cc == 0),
                            stop=(cc == KC - 1),
                        )
                if PROBE != 3:
                    nc.scalar.activation(out=eq[:, sl], in_=psq[:, :], func=AF.Exp,
                                         scale=QSCL)

                def d_softmax():
                    if PROBE == 1:
                        return
                    # softmax-over-d denominator, broadcast within head +
                    # recip; then scale eq in place (q softmax) accumulating
                    # row sums r'
                    psd = pp.tile([128, SLICE], f32, tag="ps", name="psd")
                    nc.tensor.matmul(
                        psd[:, :], bones_sb[:, :], eq[:, sl], start=True, stop=True
                    )
                    recipd = rdp.tile([128, SLICE], f32, name="recipd")
                    nc.vector.reciprocal_approx_fast(out=recipd, in_=psd[:, :])
                    nc.vector.scalar_tensor_tensor(
                        out=eq[:, sl],
                        in0=eq[:, sl],
                        scalar=1.0,
                        in1=recipd,
                        op0=OP.bypass,
                        op1=OP.mult,
                        accum_out=racc[:, j : j + 1],
                    )

                if not AORD:
                    d_softmax()

                # k,v transposed projections: x sub-slices as stationary operand
                for h in (range(0) if PROBE == 2 else range(2)):
                    pskv = pp.tile([128, SLICE], f32, tag="ps", name="pskv")
                    for s_ in range(2):
                        isub = 2 * h + s_
                        xsl = slice(isub * 128, (isub + 1) * 128)
                        po = pskv[:, s_ * 256 : (s_ + 1) * 256]
                        if QKV8R:
                            terms = [(t["x8"], wkv8_sb), (t["dx"], wkv8_sb),
                                     (t["x8"], dwkv_sb)]
                            n = 0
                            for xt, wt in terms:
                                for cc in (0, 2):
                                    nc.tensor.matmul(
                                        po,
                                        xt[:, cc : cc + 2, xsl],
                                        wt[:, cc : cc + 2, :],
                                        start=(n == 0),
                                        stop=(n == 5),
                                        perf_mode=DR,
                                    )
                                    n += 1
                        elif KV8:
                            for cc in (0, 2):
                                nc.tensor.matmul(
                                    po,
                                    t["x8"][:, cc : cc + 2, xsl],
                                    wkv8_sb[:, cc : cc + 2, :],
                                    start=(cc == 0),
                                    stop=(cc == 2),
                                    perf_mode=DR,
                                )
                        else:
                            for cc in range(KC):
                                nc.tensor.matmul(
                                    po,
                                    t["xs"][:, cc, xsl],
                                    wkv_sb[:, cc, :],
                                    start=(cc == 0),
                                    stop=(cc == KC - 1),
                                )
                    kvv = pskv[:, :].rearrange("p (s o) -> p s o", s=2)
                    a0 = 4 * j + 2 * h
                    nc.scalar.activation(
                        out=ekt[:, a0 : a0 + 2, :], in_=kvv[:, :, 0:128],
                        func=AF.Exp, scale=KSCL,
                    )
                    if VTASPLIT and h == 0:
                        nc.scalar.activation(
                            out=vta[:, a0 : a0 + 2, 0:128],
                            in_=kvv[:, :, 128:256], func=AF.Copy,
                        )
                    else:
                        veng = nc.gpsimd if VTAPOOL else nc.vector
                        veng.tensor_copy(
                            out=vta[:, a0 : a0 + 2, 0:128], in_=kvv[:, :, 128:256]
                        )
                if AORD:
                    d_softmax()

            def phase_bcd_head(b, S):
                eq, ekt, vta, racc = S["eq"], S["ekt"], S["vta"], S["racc"]

                if PROBE == 6:
                    foldT = foldp.tile([128, C], f32r, name="foldT")
                    mzT = smalls.tile([128, HID], f32r, tag="mzT", bufs=2, name="mzT")
                    v01 = smalls.tile([128, 4], f32r, tag="v01", bufs=2, name="v01")
                    nc.vector.memset(foldT.bitcast(f32), 0.0)
                    nc.vector.memset(mzT.bitcast(f32), 0.0)
                    nc.vector.memset(v01.bitcast(f32), 0.0)
                    S["foldT"] = foldT
                    S["mzT"] = mzT
                    S["v01"] = v01
                    return
                # context (accumulate over all 32 spatial chunks)
                psctx = ppctx.tile([128, SLICE], f32, name="psctx")
                for i in range(32):
                    nc.tensor.matmul(
                        psctx[:, 0:vtw_a],
                        ekt[:, i, :],
                        vta[:, i, :],
                        start=(i == 0),
                        stop=(i == 31),
                    )
                inv_sk = smalls.tile([128, 1], f32, name="inv_sk")
                nc.vector.reciprocal_approx_fast(out=inv_sk, in_=psctx[:, 128:129])
                ctx2 = smalls.tile([128, HID], f32r, tag="ctx2", bufs=2, name="ctx2")
                nc.vector.scalar_tensor_tensor(
                    out=ctx2,
                    in0=psctx[:, 0:HID],
                    scalar=inv_sk,
                    in1=mask_sb[:, :],
                    op0=OP.mult,
                    op1=OP.mult,
                )

                # transpose ctx2 on PE, then fold: foldT = ctx2T-contract wout,
                # [MM^T | v0 | v1] = ctx2T-contract [L | u0 | u1]
                psT = ppcd.tile([128, HID], f32r, tag="pscd", name="psT")
                nc.tensor.transpose(psT[:, :], ctx2[:, :], ident_sb[:, :])
                ctx2T = smalls.tile([128, HID], f32r, tag="ctx2T", bufs=2, name="ctx2T")
                nc.vector.tensor_copy(out=ctx2T, in_=psT[:, :])

                psf = ppcd.tile([128, SLICE], f32, tag="pscd", name="psf")
                nc.tensor.matmul(
                    psf[:, :], ctx2T[:, :], wout_sb[:, :], start=True, stop=True
                )
                foldT = foldp.tile([128, C], f32r, name="foldT")
                nc.scalar.activation(out=foldT, in_=psf[:, :], func=AF.Copy)

                psf2 = ppcd.tile([128, SLICE], f32, tag="pscd", name="psf2")
                nc.tensor.matmul(
                    psf2[:, 0:LMW], ctx2T[:, :], lmu_sb[:, :], start=True, stop=True
                )
                mzT = smalls.tile([128, HID], f32r, tag="mzT", bufs=2, name="mzT")
                nc.vector.tensor_copy(out=mzT, in_=psf2[:, 0:HID])
                v01 = smalls.tile([128, 4], f32r, tag="v01", bufs=2, name="v01")
                nc.vector.tensor_copy(out=v01, in_=psf2[:, HID : HID + 4])
                S["foldT"] = foldT
                S["mzT"] = mzT
                S["v01"] = v01

            def phase_bcd_z(b, S):
                eq, mzT = S["eq"], S["mzT"]
                zacc = smalls.tile([128, NSL], f32, tag="zacc", bufs=2, name="zacc")
                if PROBE == 4:
                    nc.vector.memset(zacc, 0.0)
                    S["zacc"] = zacc
                    return
                # z = MM @ eq'; accumulate ||z||^2 via in-place Square on PSUM
                for j in range(NSL):
                    sl = slice(j * SLICE, (j + 1) * SLICE)
                    psz = ppcd.tile([128, SLICE], f32, tag="pscd", name="psz")
                    nc.tensor.matmul(
                        psz[:, :], mzT[:, :], eq[:, sl], start=True, stop=True
                    )
                    nc.scalar.activation(
                        out=psz[:, :], in_=psz[:, :], func=AF.Square,
                        accum_out=zacc[:, j : j + 1],
                    )
                S["zacc"] = zacc

            def phase_bcd_stats(b, S):
                racc, zacc, v01 = S["racc"], S["zacc"], S["v01"]
                r_sb = smalls.tile([128, 1], f32, name="r_sb")
                nc.vector.tensor_reduce(out=r_sb, in_=racc, axis=AX.X, op=OP.add)
                zred = smalls.tile([128, 1], f32, name="zred")
                nc.vector.tensor_reduce(out=zred, in_=zacc, axis=AX.X, op=OP.add)
                # duplicate into even-width f32r stationary operands
                rr = smalls.tile([128, 2], f32r, name="rr")
                nc.vector.tensor_copy(out=rr[:, 0:1], in_=r_sb)
                nc.vector.tensor_copy(out=rr[:, 1:2], in_=r_sb)
                zz = smalls.tile([128, 2], f32r, name="zz")
                nc.vector.tensor_copy(out=zz[:, 0:1], in_=zred)
                nc.vector.tensor_copy(out=zz[:, 1:2], in_=zred)

                # partition-0 dots: psA[0, 0]=<v0, r'>, psA[0, 1]=<v1, r'>
                psA = ppcd.tile([128, SLICE], f32, tag="pscd", name="psA")
                nc.tensor.matmul(
                    psA[0:2, 0:4], rr[:, :], v01[:, :], start=True, stop=True
                )
                # psB[0, 0] = sum_p zred = ||z||^2 total
                psB = ppcd.tile([128, SLICE], f32, tag="pscd", name="psB")
                nc.tensor.matmul(
                    psB[0:2, 0:4], zz[:, :], ones_sb[:, :], start=True, stop=True
                )

                # scalar chain on partition 0 (fused):
                st = stp.tile([1, 16], f32, name="st")
                inv_n = 1.0 / (C * NSP)
                nc.vector.scalar_tensor_tensor(
                    out=st[:, 2:3], in0=psA[0:1, 0:1], scalar=inv_n,
                    in1=sc_sb[:, 4:5], op0=OP.mult, op1=OP.add,
                )
                nc.vector.scalar_tensor_tensor(
                    out=st[:, 0:1], in0=psA[0:1, 1:2], scalar=2.0 * inv_n,
                    in1=sc_sb[:, 5:6], op0=OP.mult, op1=OP.add,
                )
                nc.vector.scalar_tensor_tensor(
                    out=st[:, 1:2], in0=psB[0:1, 0:1], scalar=inv_n,
                    in1=st[:, 0:1], op0=OP.mult, op1=OP.add,
                )
                nc.vector.tensor_mul(out=st[:, 3:4], in0=st[:, 2:3], in1=st[:, 2:3])
                nc.vector.tensor_sub(out=st[:, 4:5], in0=st[:, 1:2], in1=st[:, 3:4])
                # rstd = rsqrt(se), DVE-only: bit-trick seed + 3 Newton steps
                # (avoids Ln/Sqrt ACT table-set switches; se > 0 always)
                i32 = mybir.dt.int32
                nc.vector.tensor_scalar(
                    out=st[:, 6:7].bitcast(i32), in0=st[:, 4:5].bitcast(i32),
                    scalar1=1, scalar2=None, op0=OP.arith_shift_right,
                )
                nc.vector.tensor_scalar(
                    out=st[:, 6:7].bitcast(i32), in0=st[:, 6:7].bitcast(i32),
                    scalar1=-1, scalar2=0x5F3759DF,
                    op0=OP.mult, op1=OP.add,
                )
                for it in range(3):
                    dst = st[:, 8:9] if it == 2 else st[:, 6:7]
                    nc.vector.tensor_mul(out=st[:, 10:11], in0=st[:, 6:7], in1=st[:, 6:7])
                    nc.vector.tensor_mul(out=st[:, 10:11], in0=st[:, 10:11], in1=st[:, 4:5])
                    nc.vector.tensor_scalar(
                        out=st[:, 10:11], in0=st[:, 10:11],
                        scalar1=-0.5, scalar2=1.5, op0=OP.mult, op1=OP.add,
                    )
                    nc.vector.tensor_mul(out=dst, in0=st[:, 6:7], in1=st[:, 10:11])
                nc.vector.tensor_mul(out=st[:, 9:10], in0=st[:, 2:3], in1=st[:, 8:9])

                # broadcast (rstd, mu*rstd) across partitions via a K=1 matmul
                stb = stp.tile([1, 2], f32r, name="stb")
                nc.vector.tensor_copy(out=stb, in_=st[:, 8:10])
                psb = ppcd.tile([128, SLICE], f32, tag="pscd", name="psb")
                nc.tensor.matmul(
                    psb[:, 0:2], onesr_sb[:, :], stb[:, :], start=True, stop=True
                )
                rb = smalls.tile([128, 2], f32, name="rb")
                nc.vector.tensor_copy(out=rb, in_=psb[:, 0:2])

                # A = gamma*rstd ; Bc = gb*rstd - gamma*(mu*rstd) + beta
                a4 = smalls.tile([128, MC], f32, tag="a4", bufs=2, name="a4")
                nc.vector.tensor_scalar_mul(out=a4, in0=g4_sb[:, :], scalar1=rb[:, 0:1])
                b4 = smalls.tile([128, MC], f32, tag="b4", bufs=2, name="b4")
                nc.vector.tensor_scalar_mul(out=b4, in0=gb4_sb[:, :], scalar1=rb[:, 0:1])
                t4 = smalls.tile([128, MC], f32, name="t4")
                nc.vector.tensor_scalar_mul(out=t4, in0=g4_sb[:, :], scalar1=rb[:, 1:2])
                nc.vector.tensor_sub(out=b4, in0=b4, in1=t4)
                nc.vector.tensor_add(out=b4, in0=b4, in1=be4_sb[:, :])
                S["a4"] = a4
                S["b4"] = b4

            def phase_bcd(b, S):
                phase_bcd_head(b, S)
                phase_bcd_z(b, S)
                phase_bcd_stats(b, S)

            def _ycopy(dst, psy, a4, b4, m, idx):
                if YSPLIT == 3:
                    eng = 1 if (m == 3) else 0
                elif YSPLIT == 2:
                    eng = idx % 3
                else:
                    eng = (m % 2) if YSPLIT else 0
                if eng == 1:
                    nc.vector.tensor_scalar(
                        out=dst, in0=psy[:, :],
                        scalar1=a4[:, m : m + 1], scalar2=b4[:, m : m + 1],
                        op0=OP.mult, op1=OP.add,
                    )
                elif eng == 2:
                    nc.gpsimd.tensor_scalar(
                        out=dst, in0=psy[:, :],
                        scalar1=a4[:, m : m + 1], scalar2=b4[:, m : m + 1],
                        op0=OP.mult, op1=OP.add,
                    )
                else:
                    nc.scalar.activation(
                        out=dst, in_=psy[:, :], func=AF.Identity,
                        bias=b4[:, m : m + 1], scale=a4[:, m : m + 1],
                    )

            def phase_e(b, S, js=None):
                if PROBE == 5:
                    return
                foldT, a4, b4 = S["foldT"], S["a4"], S["b4"]
                eq = S["eq"]
                oa = out_d.ap()[b].rearrange("(mm p) n -> p mm n", p=128)
                store_eng = [nc.sync, nc.gpsimd, nc.scalar][KSTORE]
                jl = list(js if js is not None else range(NSL))
                if EMAJOR:
                    idx = 0
                    for m in range(MC):
                        for j in jl:
                            sl = slice(j * SLICE, (j + 1) * SLICE)
                            psy = ppcd.tile([128, SLICE], f32, tag="pscd", name="psy")
                            nc.tensor.matmul(
                                psy[:, :],
                                foldT[:, m * 128 : (m + 1) * 128],
                                eq[:, sl],
                                start=True,
                                stop=True,
                            )
                            ys1 = yp.tile([128, SLICE], odt, tag="ys1", name="ys1")
                            _ycopy(ys1[:, :], psy, a4, b4, m, idx)
                            idx += 1
                            if not NOSTORE:
                                store_eng.dma_start(out=oa[:, m, sl], in_=ys1)
                    return
                for j in jl:
                    sl = slice(j * SLICE, (j + 1) * SLICE)
                    ys4 = yp.tile([128, MC, SLICE], odt, name="ys4")
                    for m in range(MC):
                        psy = ppcd.tile([128, SLICE], f32, tag="pscd", name="psy")
                        nc.tensor.matmul(
                            psy[:, :],
                            foldT[:, m * 128 : (m + 1) * 128],
                            eq[:, sl],
                            start=True,
                            stop=True,
                        )
                        _ycopy(ys4[:, m, :], psy, a4, b4, m, m)
                    if not NOSTORE:
                        store_eng.dma_start(out=oa[:, :, sl], in_=ys4)

            if KPIPE == 1 and SCHED:
                # PE stays streaming through BCD: E(b-1) head while ctx/fold
                # DVE handoffs run, z(b) while squares accumulate, E(b-1)
                # tail while the serial rstd chain runs on DVE.
                states = []
                mid_consts()
                for i_b, b in enumerate(batch_seq):
                    S = phase_a_init(b, i_b)
                    states.append(S)
                    for j in range(NSL):
                        phase_a_slice(S, j)
                    prev = states[i_b - 1] if i_b > 0 else None
                    if prev is not None:
                        phase_e(batch_seq[i_b - 1], prev, js=range(0, 4))
                    phase_bcd_head(b, S)
                    phase_bcd_z(b, S)
                    if prev is not None:
                        phase_e(batch_seq[i_b - 1], prev, js=range(4, NSL))
                    phase_bcd_stats(b, S)
                states and phase_e(batch_seq[-1], states[-1])
            elif KPIPE == 1:
                states = []
                mid_consts()
                for i_b, b in enumerate(batch_seq):
                    S = phase_a_init(b, i_b)
                    states.append(S)
                    for j in range(NSL):
                        phase_a_slice(S, j)
                    if i_b > 0:
                        phase_e(batch_seq[i_b - 1], states[i_b - 1])
                    phase_bcd(b, S)
                states and phase_e(batch_seq[-1], states[-1])
            else:
                mid_consts()
                for i_b, b in enumerate(batch_seq):
                    S = phase_a_init(b, i_b)
                    for j in range(NSL):
                        phase_a_slice(S, j)
                    phase_bcd(b, S)
                    phase_e(b, S)

    nc.compile()
    return nc


def _host_consts(W_qkv, W_out, b_out, gamma, beta, cfg=None):
    import ml_dtypes

    cfg = dict(cfg or env_cfg())
    FP8KV = cfg["KV8"] or cfg["QKV8R"]
    QKV8R = cfg["QKV8R"]
    X3 = cfg["X3"]

    W_qkv = np.asarray(W_qkv, np.float32)
    W_out = np.asarray(W_out, np.float32)
    b_out = np.asarray(b_out, np.float32)
    gamma = np.asarray(gamma, np.float32)
    beta = np.asarray(beta, np.float32)

    e4 = ml_dtypes.float8_e4m3
    e5 = ml_dtypes.float8_e5m2
    e3 = ml_dtypes.float8_e3m4
    bf = ml_dtypes.bfloat16

    c = {}
    wq_t = np.ascontiguousarray(W_qkv[:HID].T)                      # [512, 128]
    wkv_t = np.ascontiguousarray(W_qkv[HID : 3 * HID].T)            # [512, 256]
    if QKV8R:
        wq16 = WS8 * wq_t
        c["wq8_t"] = wq16.astype(e4)
        c["dwq5_t"] = (wq16 - c["wq8_t"].astype(np.float32)).astype(e5)
        wkv16 = WS8 * wkv_t
        c["wkv8_t"] = wkv16.astype(e4)
        c["dwkv5_t"] = (wkv16 - c["wkv8_t"].astype(np.float32)).astype(e5)
    else:
        if cfg["XCAST"] == 2 and not X3:
            c["wq_t"] = wq_t.astype(bf)
        else:
            c["wq_t"] = wq_t.astype(bf) if X3 else wq_t
        if cfg["KV8"]:
            c["wkv8_t"] = (WS8 * wkv_t).astype(e4)
        elif cfg["XCAST"] == 2 or X3:
            c["wkv_t"] = wkv_t.astype(bf)
        else:
            c["wkv_t"] = wkv_t
    c["wout_t"] = np.ascontiguousarray(W_out.T)                     # [128, 512]
    blk = np.kron(np.eye(HEADS, dtype=np.float32), np.ones((DH, DH), np.float32))
    c["b_ones"] = blk.astype(bf) if cfg["EQBF"] else blk            # [128, 128]
    mscale = SCALE / WS8 if FP8KV else SCALE
    c["mask_scale"] = (blk * mscale).astype(np.float32)
    G = (W_out.astype(np.float64).T @ W_out.astype(np.float64))
    try:
        L = np.linalg.cholesky(G)
    except np.linalg.LinAlgError:
        w_ev, V = np.linalg.eigh(G)
        L = V @ np.diag(np.sqrt(np.clip(w_ev, 0.0, None)))
    LMW = 256 if cfg["LMUPAD"] else 132
    lmu = np.zeros((HID, LMW), np.float32)
    lmu[:, 0:HID] = L.astype(np.float32)
    lmu[:, HID] = W_out.sum(axis=0)
    lmu[:, HID + 1] = W_out.T @ b_out
    c["lmu"] = lmu
    c["ident"] = np.eye(HID, dtype=np.float32)
    c["ones_col"] = np.ones((HID, 4), np.float32)
    c["ones_row"] = np.ones((1, HID), np.float32)
    vones = np.ones((128, 32, VTW - 128), np.float32)
    c["vones"] = vones.astype(bf) if cfg["CTXBF16"] else vones
    c["gamma4"] = np.ascontiguousarray(gamma.reshape(MC, 128).T)
    c["gb4"] = np.ascontiguousarray((gamma * b_out).reshape(MC, 128).T)
    c["beta4"] = np.ascontiguousarray(beta.reshape(MC, 128).T)
    ntot = float(C) * float(NSP)
    sc = np.zeros((1, 8), np.float32)
    c1 = NSP * float(b_out.astype(np.float64).sum())
    c2 = NSP * float((b_out.astype(np.float64) ** 2).sum())
    sc[0, 0] = c1
    sc[0, 1] = c2
    sc[0, 2] = EPS
    sc[0, 3] = 1.0 / ntot
    sc[0, 4] = c1 / ntot                         # C1*invN
    sc[0, 5] = c2 / ntot + EPS                   # C2*invN + eps
    c["sc"] = sc
    return c


def _host_x(x, cfg=None):
    """Full f32 x -> per-core dict of [BPC, C, NSP] shards."""
    import ml_dtypes

    cfg = dict(cfg or env_cfg())
    x = np.asarray(x, np.float32)
    xr = x.reshape(B, C, NSP)

    e4 = ml_dtypes.float8_e4m3
    e5 = ml_dtypes.float8_e5m2
    e3 = ml_dtypes.float8_e3m4
    bf = ml_dtypes.bfloat16

    full = {}
    if cfg["QKV8R"]:
        x8 = xr.astype(e4)
        full["x8"] = x8
        full["dx5"] = (xr - x8.astype(np.float32)).astype(e5)
    else:
        if cfg["X3"]:
            full["x"] = xr.astype(e3)
        elif cfg["XCAST"]:
            full["x"] = xr.astype(bf)
        else:
            full["x"] = xr
        if cfg["KV8"]:
            full["x8"] = xr.astype(e4)
    out = []
    for ci in range(NCORES):
        out.append({
            k: np.ascontiguousarray(v[ci * BPC : (ci + 1) * BPC])
            for k, v in full.items()
        })
    return out


def make_in_maps(inputs, cfg=None):
    cfg = dict(cfg or env_cfg())
    consts = _host_consts(
        inputs["W_qkv"], inputs["W_out"], inputs["b_out"],
        inputs["gamma"], inputs["beta"], cfg,
    )
    xs = _host_x(inputs["x"], cfg)
    in_maps = []
    for ci in range(NCORES):
        m = dict(xs[ci])
        m.update(consts)
        in_maps.append(m)
    return in_maps


def kernel(x, W_qkv, W_out, b_out, gamma, beta):
    from concourse.bass_utils import run_bass_kernel_spmd

    cfg = env_cfg()
    x = np.asarray(x, np.float32)
    assert x.shape == (B, C, HGT, WID)
    key = tuple(sorted(cfg.items()))
    if key not in _CACHE:
        _CACHE[key] = _build_nc(cfg=cfg)
    nc = _CACHE[key]

    in_maps = make_in_maps(
        dict(x=x, W_qkv=W_qkv, W_out=W_out, b_out=b_out, gamma=gamma, beta=beta),
        cfg,
    )
    res = run_bass_kernel_spmd(nc, in_maps, core_ids=list(range(NCORES)))
    out = np.concatenate([np.asarray(r_["out"], np.float32) for r_ in res.results], axis=0)
    return np.ascontiguousarray(out.reshape(B, C, HGT, WID))
